# revision 1
# baseline (speedup 1.0000x reference)
"""DCNv2 (deformable conv v2) Trainium2 kernel — 8-core SPMD, batch x H-half sharding.

Self-contained: host-side layout prep (permutation/cast only) + Bass/Tile device
program. All FLOP-bearing compute (offset conv, bilinear sampling via SWDGE
gather + PE diag-blend, main GEMM) runs on the NeuronCores.
"""
import numpy as np
from contextlib import ExitStack

import concourse.bass as bass
import concourse.mybir as mybir
import concourse.tile as tile
from concourse import bacc
from concourse.bass_utils import run_bass_kernel_spmd

F16 = np.float16

B, C, H, W = 4, 256, 64, 64
O = 256
K = 3
KK = 9
NCORES = 8
ROWS_PER_CORE = 32
PIX_PER_CORE = ROWS_PER_CORE * W
PH_ROWS = 16
PH_PIX = PH_ROWS * W
XT_ROWS = 65 + H * W + 67  # 4228
PAD66 = 66
P66 = 66

f16 = mybir.dt.float16
f32 = mybir.dt.float32
i16 = mybir.dt.int16
Alu = mybir.AluOpType
Act = mybir.ActivationFunctionType

def host_prep(x, weight, bias, w_om, b_om):
    """Build all per-core input tensors. x:[B,C,H,W] f32, weight:[O,C,3,3],
    bias:[O], w_om:[27,C,3,3], b_om:[27]."""
    # weight reorder to contraction index (k, ch): row = k*256 + ch
    # wT_sb[p, t*256 + o] = weight[o, ch, ky, kx],  t = k*2 + chh, ch = chh*128 + p
    wr = weight.reshape(O, C, KK).transpose(2, 1, 0).reshape(KK * C, O)  # [(k,ch), O]
    wT_sb = wr.reshape(18, 128, O).transpose(1, 0, 2).reshape(128, 18 * O).astype(F16)
    womr = w_om.reshape(27, C, KK).transpose(2, 1, 0).reshape(KK * C, 27)
    womT_sb = womr.reshape(18, 128, 27).transpose(1, 0, 2).reshape(128, 18 * 27).astype(F16)
    bias_sb = bias.reshape(2, 128).T.astype(np.float32)         # [128, 2]
    bom_sb = b_om.reshape(27, 1).astype(np.float32)             # [27, 1]
    ident16 = np.eye(128, dtype=F16)

    per_core = []
    for core in range(NCORES):
        b, h = divmod(core, 2)
        xb = x[b]  # [C, H, W]
        # xt: pixel-major padded f16 [XT_ROWS, 256]
        xt = np.zeros((XT_ROWS, C), dtype=F16)
        xt[65:65 + H * W, :] = xb.reshape(C, H * W).T.astype(F16)
        # xpad16: 66x66 zero-padded channel-major f16, rows [32h, 32h+34) of the
        # padded frame (covers the core's 32 output rows + 3x3 halo), 2 ch tiles
        xp = np.zeros((C, PAD66, PAD66), dtype=F16)
        xp[:, 1:65, 1:65] = xb.astype(F16)
        xpc = xp[:, 32 * h:32 * h + 34, :]
        xpad16 = np.ascontiguousarray(xpc).reshape(2, 128, 34 * PAD66)
        # grids
        part = np.arange(128)
        kk = np.arange(KK)
        ky, kx = kk // K, kk % K
        # raster pixel enumeration: j = 64 r + c ; part = j%128 = 64(r%2)+c, slot = r//2
        r2 = part // 64          # r % 2
        ccol = part % 64         # c
        slot = np.arange(8)
        # gyk[p, ph, slot, k] = 32h + 16 ph + (2 slot + r2) - 1 + ky
        gyk = np.zeros((128, 2, 8, KK), dtype=np.float32)
        for ph in range(2):
            gyk[:, ph, :, :] = ((32 * h + 16 * ph + r2 - 1)[:, None, None]
                                + 2 * slot[None, :, None] + ky[None, None, :])
        # gxk[p, slot, k] = c - 1 + kx (slot-independent)
        gxk = np.broadcast_to((ccol - 1)[:, None, None] + kx[None, None, :],
                              (128, 8, KK)).astype(np.float32).copy()
        per_core.append(dict(
            xt=xt, xpad16_0=xpad16[0].copy(), xpad16_1=xpad16[1].copy(),
            wT=wT_sb, womT=womT_sb, bias=bias_sb, bom=bom_sb,
            gyk=gyk.reshape(128, 2 * 8 * KK), gxk=gxk.reshape(128, 8 * KK),
            ident=ident16,
        ))
    return per_core



def ap_of(base, offset_delta, dims):
    """Manual AP on the same tensor: dims = list of [step, count] (free dims).
    Keeps the partition dim of `base`."""
    return bass.AP(tensor=base.tensor, offset=base.offset + offset_delta,
                   ap=[base.ap[0]] + dims)


def build_nc(skip_compile=False):
    nc = bacc.Bacc("TRN2", target_bir_lowering=False, debug=False, num_devices=8)

    xt_d = nc.dram_tensor("xt", [XT_ROWS, 256], f16, kind="ExternalInput")
    xp_d = [nc.dram_tensor(f"xp{i}", [128, 34, P66], f16, kind="ExternalInput")
            for i in range(2)]
    wT_d = nc.dram_tensor("wT", [128, 18 * 256], f16, kind="ExternalInput")
    womT_d = nc.dram_tensor("womT", [128, 18 * 27], f16, kind="ExternalInput")
    bias_d = nc.dram_tensor("bias", [128, 2], f32, kind="ExternalInput")
    bom_d = nc.dram_tensor("bom", [27, 1], f32, kind="ExternalInput")
    gyk_d = nc.dram_tensor("gyk", [128, 2, 8, KK], f32, kind="ExternalInput")
    gxk_d = nc.dram_tensor("gxk", [128, 8, KK], f32, kind="ExternalInput")
    ident_d = nc.dram_tensor("ident", [128, 128], f16, kind="ExternalInput")
    out_d = nc.dram_tensor("out", [256, 2048], f32, kind="ExternalOutput")

    # overlapping pair view of xt for the gather: row i -> 512 contiguous f16
    xt_pairs = bass.AP(tensor=xt_d, offset=0, ap=[[256, XT_ROWS - 1], [1, 512]])

    with tile.TileContext(nc) as tc, ExitStack() as ctx:
        const = ctx.enter_context(tc.tile_pool(name="const", bufs=1))
        xp = [const.tile([128, 34, P66], f16, name=f"xp{i}", tag=f"xp{i}") for i in range(2)]
        wT = const.tile([128, 18 * 256], f16, name="wT", tag="wT")
        womT = const.tile([128, 18 * 27], f16, name="womT", tag="womT")
        bias_t = const.tile([128, 2], f32, name="bias", tag="bias")
        bom_t = const.tile([27, 1], f32, name="bom", tag="bom")
        gyk_t = const.tile([128, 2, 8, KK], f32, name="gyk", tag="gyk")
        gxk_t = const.tile([128, 8, KK], f32, name="gxk", tag="gxk")
        ident_t = const.tile([128, 128], f16, name="ident", tag="ident")
        for t_, d_ in ((xp[0], xp_d[0]), (xp[1], xp_d[1]), (womT, womT_d),
                       (gyk_t, gyk_d), (gxk_t, gxk_d), (bom_t, bom_d),
                       (ident_t, ident_d), (wT, wT_d), (bias_t, bias_d)):
            nc.sync.dma_start(out=t_[:], in_=d_.ap())

        # psum pools: 1 + 1 + 2 + 2*2 = 8 banks total
        omp_pool = ctx.enter_context(tc.tile_pool(name="omp", bufs=1, space="PSUM"))
        # PE warm-up train: ~4us of tiny matmuls from t=0 so the HAM clock gate
        # releases (1.2 -> 2.4 GHz) before the om GEMM issues on real HW.
        warm_src = const.tile([128, 128], f16, name="warm_src", tag="warm_src")
        nc.gpsimd.memset(warm_src[:], 0.0)
        warm_ps = omp_pool.tile([128, 512], f32, name="warm_ps", tag="psum_om")
        for wi in range(40):
            nc.tensor.matmul(warm_ps[:, (wi % 4) * 128:(wi % 4) * 128 + 128],
                             lhsT=warm_src[:], rhs=warm_src[:],
                             start=True, stop=True)
        omtp_pool = ctx.enter_context(tc.tile_pool(name="omtp", bufs=1, space="PSUM"))
        colp_pool = ctx.enter_context(tc.tile_pool(name="colp", bufs=2, space="PSUM"))
        outp_pool = ctx.enter_context(tc.tile_pool(name="outp", bufs=1, space="PSUM"))

        omsb_pool = ctx.enter_context(tc.tile_pool(name="omsb", bufs=2))
        math_pool = ctx.enter_context(tc.tile_pool(name="math", bufs=2))
        idxw_pool = ctx.enter_context(tc.tile_pool(name="idxw", bufs=2))
        g_pool = ctx.enter_context(tc.tile_pool(name="g", bufs=6))
        diag_pool = ctx.enter_context(tc.tile_pool(name="diag", bufs=48))
        cols_pool = ctx.enter_context(tc.tile_pool(name="cols", bufs=6))
        outs_pool = ctx.enter_context(tc.tile_pool(name="outs", bufs=2))

        for ph in range(2):
            # ---------- om conv GEMM -> om_sb [27, 1024] f16 (+bias) ----------
            om_sb = omsb_pool.tile([27, PH_PIX], f16, name="om_sb", tag="om_sb")
            for n5 in range(2):
                psum_om = omp_pool.tile([27, 512], f32, name="psum_om", tag="psum_om")
                for t in range(18):
                    k, chh = divmod(t, 2)
                    ky, kx = k // 3, k % 3
                    r0 = 16 * ph + ky + n5 * 8
                    rhs = xp[chh][:, r0:r0 + 8, kx:kx + 64]  # raster (r outer, c inner)
                    nc.tensor.matmul(
                        psum_om[:], lhsT=womT[:, t * 27:(t + 1) * 27], rhs=rhs,
                        start=(t == 0), stop=(t == 17))
                nc.scalar.activation(om_sb[:, n5 * 512:(n5 + 1) * 512], psum_om[:],
                                     Act.Identity, bias=bom_t[:])

            # ---------- om transpose to pixel-major ompm [128, 8, 27] f32 ----------
            ompm = math_pool.tile([128, 8, 27], f32, name="ompm", tag="ompm")
            pom = omtp_pool.tile([128, 8, 27], f32, name="pom", tag="pom")
            for q in range(8):
                nc.tensor.matmul(pom[:, q], lhsT=om_sb[:, q * 128:(q + 1) * 128],
                                 rhs=ident_t[0:27, 0:27], start=True, stop=True)
            nc.scalar.activation(ompm[:], pom[:], Act.Copy)

            # ---------- offset math (pixel-major [128, 8, 9] f32) ----------
            def mt(tag):
                return math_pool.tile([128, 8, KK], f32, name=tag, tag=tag)

            dy = ompm[:, :, 0:KK]
            dx = ompm[:, :, KK:2 * KK]
            ml = ompm[:, :, 2 * KK:3 * KK]
            ty, tx_, fry, frx = mt("ty"), mt("tx"), mt("fry"), mt("frx")
            y064, x064, m_t = mt("y064"), mt("x064"), mt("m")
            s0y, g0, ay0, ay0m = mt("s0y"), mt("g0"), mt("ay0"), mt("ay0m")
            g1, ay1, ay1m = mt("g1"), mt("ay1"), mt("ay1m")
            s0x, g0x, bx0, g1x, bx1 = mt("s0x"), mt("g0x"), mt("bx0"), mt("g1x"), mt("bx1")
            wc = [mt(f"wc{i}") for i in range(4)]
            ya, xa, idxf, idxf0, idxf1 = mt("ya"), mt("xa"), mt("idxf"), mt("if0"), mt("if1")

            V = nc.vector
            V.tensor_tensor(ty[:], dy, gyk_t[:, ph], Alu.add)
            V.tensor_scalar_add(ty[:], ty[:], 64.0)
            V.tensor_tensor(tx_[:], dx, gxk_t[:], Alu.add)
            V.tensor_scalar_add(tx_[:], tx_[:], 64.0)
            # floor via int-cast roundtrip (exact for any HW rounding mode; ty>0)
            yi32 = math_pool.tile([128, 8, KK], mybir.dt.int32, name="yi32", tag="yi32")
            xi32 = math_pool.tile([128, 8, KK], mybir.dt.int32, name="xi32", tag="xi32")
            yif, xif = mt("yif"), mt("xif")
            gq, gqx = mt("gq"), mt("gqx")
            V.tensor_copy(yi32[:], ty[:])
            V.tensor_copy(yif[:], yi32[:])
            V.tensor_tensor(gq[:], yif[:], ty[:], Alu.is_gt)
            V.tensor_tensor(y064[:], yif[:], gq[:], Alu.subtract)
            V.tensor_tensor(fry[:], ty[:], y064[:], Alu.subtract)
            V.tensor_copy(xi32[:], tx_[:])
            V.tensor_copy(xif[:], xi32[:])
            V.tensor_tensor(gqx[:], xif[:], tx_[:], Alu.is_gt)
            V.tensor_tensor(x064[:], xif[:], gqx[:], Alu.subtract)
            V.tensor_tensor(frx[:], tx_[:], x064[:], Alu.subtract)
            nc.scalar.activation(m_t[:], ml, Act.Sigmoid)
            V.tensor_scalar(s0y[:], fry[:], -1.0, 1.0, Alu.mult, Alu.add)
            V.scalar_tensor_tensor(g0[:], y064[:], 64.0, s0y[:], Alu.is_ge, Alu.mult)
            V.scalar_tensor_tensor(ay0[:], y064[:], 127.0, g0[:], Alu.is_le, Alu.mult)
            V.tensor_tensor(ay0m[:], ay0[:], m_t[:], Alu.mult)
            V.scalar_tensor_tensor(g1[:], y064[:], 63.0, fry[:], Alu.is_ge, Alu.mult)
            V.scalar_tensor_tensor(ay1[:], y064[:], 126.0, g1[:], Alu.is_le, Alu.mult)
            V.tensor_tensor(ay1m[:], ay1[:], m_t[:], Alu.mult)
            V.tensor_scalar(s0x[:], frx[:], -1.0, 1.0, Alu.mult, Alu.add)
            V.scalar_tensor_tensor(g0x[:], x064[:], 64.0, s0x[:], Alu.is_ge, Alu.mult)
            V.scalar_tensor_tensor(bx0[:], x064[:], 127.0, g0x[:], Alu.is_le, Alu.mult)
            V.scalar_tensor_tensor(g1x[:], x064[:], 63.0, frx[:], Alu.is_ge, Alu.mult)
            V.scalar_tensor_tensor(bx1[:], x064[:], 126.0, g1x[:], Alu.is_le, Alu.mult)
            V.tensor_tensor(wc[0][:], ay0m[:], bx0[:], Alu.mult)
            V.tensor_tensor(wc[1][:], ay0m[:], bx1[:], Alu.mult)
            V.tensor_tensor(wc[2][:], ay1m[:], bx0[:], Alu.mult)
            V.tensor_tensor(wc[3][:], ay1m[:], bx1[:], Alu.mult)
            V.tensor_scalar(ya[:], y064[:], 63.0, 127.0, Alu.max, Alu.min)
            V.tensor_scalar(xa[:], x064[:], 63.0, 127.0, Alu.max, Alu.min)
            V.scalar_tensor_tensor(idxf[:], ya[:], 64.0, xa[:], Alu.mult, Alu.add)
            V.tensor_scalar_add(idxf0[:], idxf[:], -4095.0)
            V.tensor_scalar_add(idxf1[:], idxf[:], -4031.0)
            # layout (k, rc, v) so the wrapped-rearrange DMA source is contiguous
            idx_pm = math_pool.tile([128, KK, 2, 8], i16, name="idx_pm", tag="idx_pm")
            ipb = idx_pm[:]
            cast0 = bass.AP(tensor=ipb.tensor, offset=ipb.offset,
                            ap=[ipb.ap[0], [1, 8], [16, KK]])
            cast1 = bass.AP(tensor=ipb.tensor, offset=ipb.offset + 8,
                            ap=[ipb.ap[0], [1, 8], [16, KK]])
            V.tensor_copy(cast0, idxf0[:])
            V.tensor_copy(cast1, idxf1[:])

            # ---------- idx rearrange to wrapped layout idxw [128, 9, 2, 64] ----------
            idxw = idxw_pool.tile([128, KK, 2, 64], i16, name="idxw", tag="idxw")
            for u in range(8):
                r2, cc = divmod(u, 4)
                s = idx_pm[64 * r2 + 16 * cc: 64 * r2 + 16 * cc + 16]
                src = bass.AP(tensor=s.tensor, offset=s.offset,
                              ap=[s.ap[0], [1, KK * 2 * 8]])
                d = idxw[0:16]
                dst = bass.AP(tensor=d.tensor, offset=d.offset + 4 * r2 + cc,
                              ap=[d.ap[0], [128, KK], [64, 2], [8, 8]])
                [nc.scalar, nc.sync][u % 2].dma_start(out=dst, in_=src)
            rep_src = idxw[0:16]
            rep_src = bass.AP(tensor=rep_src.tensor, offset=rep_src.offset,
                              ap=[rep_src.ap[0], [1, KK * 2 * 64]])
            for g in range(1, 8):
                d = idxw[16 * g:16 * g + 16]
                dst = bass.AP(tensor=d.tensor, offset=d.offset,
                              ap=[d.ap[0], [1, KK * 2 * 64]])
                [nc.scalar, nc.sync][g % 2].dma_start(out=dst, in_=rep_src)

            # ---------- per-tap: gather, diag blend, main GEMM ----------
            psum_out = [outp_pool.tile([128, PH_PIX], f32, name=f"po{o2}", tag=f"po{o2}")
                        for o2 in range(2)]
            for k in range(KK):
                G = [g_pool.tile([128, 8, 512], f16, name=f"G{rc}", tag=f"G{rc}") for rc in range(2)]
                for rc in range(2):
                    nc.gpsimd.dma_gather(
                        G[rc][:], xt_pairs, idxw[:, k, rc, :], PH_PIX, PH_PIX,
                        elem_size=512, elem_step=256,
                        queue_num=0)
                # build the 4 diag weight tiles once per (k, q), reused by both chh
                diags = []
                for q in range(8):
                    d4 = []
                    for c4 in range(4):
                        diag = diag_pool.tile([128, 128], f16, name="diag", tag="diag")
                        wsl = wc[c4][:, q, k:k + 1]
                        nc.vector.tensor_scalar(diag[:], ident_t[:], wsl, None, Alu.mult)
                        d4.append(diag)
                    diags.append(d4)
                for chh in range(2):
                    cols = cols_pool.tile([128, PH_PIX], f16, name="cols", tag="cols")
                    for qh in range(2):
                        pc = colp_pool.tile([128, 512], f32, name="pc", tag="pc")
                        for qq in range(4):
                            q = qh * 4 + qq
                            for c4 in range(4):
                                rc, xc = divmod(c4, 2)
                                nc.tensor.matmul(
                                    pc[:, qq * 128:(qq + 1) * 128],
                                    lhsT=G[rc][:, q, xc * 256 + chh * 128:
                                               xc * 256 + chh * 128 + 128],
                                    rhs=diags[q][c4][:],
                                    start=(c4 == 0), stop=(c4 == 3))
                        nc.scalar.activation(cols[:, qh * 512:(qh + 1) * 512],
                                             pc[:], Act.Copy)
                    t = k * 2 + chh
                    for o2 in range(2):
                        for n5 in range(2):
                            nc.tensor.matmul(
                                psum_out[o2][:, n5 * 512:(n5 + 1) * 512],
                                lhsT=wT[:, t * 256 + o2 * 128:
                                        t * 256 + o2 * 128 + 128],
                                rhs=cols[:, n5 * 512:(n5 + 1) * 512],
                                start=(t == 0), stop=(t == 17))

            # ---------- bias + store (unpermute to raster) ----------
            for o2 in range(2):
                osb = outs_pool.tile([128, PH_PIX], f32, name=f"osb{o2}", tag=f"osb{o2}")
                nc.scalar.activation(osb[:], psum_out[o2][:], Act.Identity,
                                     bias=bias_t[:, o2:o2 + 1])
                od = out_d.ap()
                dst = bass.AP(tensor=od.tensor,
                              offset=od.offset + o2 * 128 * 2048 + ph * PH_PIX,
                              ap=[[2048, 128], [1, PH_PIX]])
                nc.sync.dma_start(out=dst, in_=osb[:])

    if not skip_compile:
        nc.compile()
    return nc


_NC_CACHE = {}


def _get_nc():
    if "nc" not in _NC_CACHE:
        _NC_CACHE["nc"] = build_nc()
    return _NC_CACHE["nc"]


def kernel(x, weight, bias, w_om, b_om):
    x = np.ascontiguousarray(np.asarray(x, dtype=np.float32))
    weight = np.asarray(weight, dtype=np.float32)
    bias = np.asarray(bias, dtype=np.float32)
    w_om = np.asarray(w_om, dtype=np.float32)
    b_om = np.asarray(b_om, dtype=np.float32)

    per_core = host_prep(x, weight, bias, w_om, b_om)
    in_maps = []
    for pc in per_core:
        in_maps.append({
            "xt": pc["xt"],
            "xp0": pc["xpad16_0"].reshape(128, 34, 66),
            "xp1": pc["xpad16_1"].reshape(128, 34, 66),
            "wT": pc["wT"], "womT": pc["womT"],
            "bias": pc["bias"], "bom": pc["bom"],
            "gyk": pc["gyk"].reshape(128, 2, 8, 9),
            "gxk": pc["gxk"].reshape(128, 8, 9),
            "ident": pc["ident"],
        })

    nc = _get_nc()
    res = run_bass_kernel_spmd(nc, in_maps, list(range(NCORES)))

    out = np.zeros((B, O, H, W), dtype=np.float32)
    for core in range(NCORES):
        b, h = divmod(core, 2)
        oc = res.results[core]["out"]
        out[b, :, 32 * h:32 * h + 32, :] = oc.reshape(O, ROWS_PER_CORE, W)
    return out

